# revision 26
# baseline (speedup 1.0000x reference)
"""Trainium2 Bass kernel for BCModel: Embedding -> LSTM -> mean/max pool -> MLP -> sigmoid.

Sharding: data-parallel over batch. B=512 split as 64 rows per core across 8 cores.
Weights/embedding table replicated. No collectives needed (forward only).

Per-core pipeline:
  1. indirect-DMA gather of embedding rows (f32, token-partition layout, one
     128-row DMA per block: HW DGE supports one offset per partition)
  2. PE transpose -> xe^T [E=128, tok] layout, evicted to bf16
  3. projection matmul xproj = W_ih^T @ xe^T + b (bf16 x bf16 -> f32), all
     timesteps up front, stored f32 interleaved [128, 2, cols] per chunk
  4. 256-step LSTM recurrence, transposed layout (partition = feature,
     free = batch), gate order [i, f, o, g]:
       - gate psum [128, 128] (one bank): cols 0:64 = [i|f], 64:128 = [o|g]
       - seeded with xproj_t by DMA, W_hh^T h accumulated on top (bf16, h bf16)
       - sigmoid over the whole rect (junk where g sits), tanh(g), tanh(c')
       - c stays f32; sum-pool accumulated on PE (identity fold), max on DVE
  5. final head: out = sigmoid(wf_avg^T sum + wf_max^T max + bf)
"""

import numpy as np

B, T, E, H, VOCAB = 512, 256, 128, 64, 50000
NCORES = 8
BL = B // NCORES          # 64 batch rows per core
P = 128
NBLK = (BL * T) // P      # 128 gather blocks of 128 tokens (2 timesteps each)
CHUNK = 16                # blocks per chunk (2048 tokens)
NCHUNK = NBLK // CHUNK    # 8
CCOLS = CHUNK * P         # 2048 xeT columns per chunk
STEPS_PER_CHUNK = T // NCHUNK  # 32

_CACHE = {}


def _build_module():
    import concourse.bass as bass
    import concourse.mybir as mybir
    import concourse.tile as tile
    from concourse import bacc
    from concourse.masks import make_identity
    from concourse.tile_rust import add_dep_helper

    fp32 = mybir.dt.float32
    bf16 = mybir.dt.bfloat16
    i32 = mybir.dt.int32
    AF = mybir.ActivationFunctionType

    nc = bacc.Bacc(None, target_bir_lowering=False, debug=False)

    with tile.TileContext(nc) as tc:
        with (
            tc.tile_pool(name="dram", bufs=1, space="DRAM") as dram,
            tc.tile_pool(name="const", bufs=1) as const,
            tc.tile_pool(name="xe_pool", bufs=2) as xe_pool,
            tc.tile_pool(name="xet_pool", bufs=2) as xet_pool,
            tc.tile_pool(name="xp_pool", bufs=1) as xp_pool,
            tc.tile_pool(name="state", bufs=1) as state,
            tc.tile_pool(name="ps_tr", bufs=2, space="PSUM") as ps_tr,
            tc.tile_pool(name="ps_pj", bufs=2, space="PSUM") as ps_pj,
            tc.tile_pool(name="ps_g", bufs=2, space="PSUM") as ps_g,
            tc.tile_pool(name="ps_pool", bufs=1, space="PSUM") as ps_pool,
        ):
            # ---- DRAM I/O ----
            xb_d = dram.tile([P, NBLK], i32, kind="ExternalInput", uniquify=False, name="xb")
            emb_d = dram.tile([VOCAB, E], bf16, kind="ExternalInput", uniquify=False, name="emb")
            wih_d = dram.tile([E, 4 * H], bf16, kind="ExternalInput", uniquify=False, name="wih")
            whh_d = dram.tile([H, 4 * H], bf16, kind="ExternalInput", uniquify=False, name="whh")
            b_d = dram.tile([2, P], fp32, kind="ExternalInput", uniquify=False, name="blstm")
            h0_d = dram.tile([H, BL], bf16, kind="ExternalInput", uniquify=False, name="h0t")
            c0_d = dram.tile([H, BL], fp32, kind="ExternalInput", uniquify=False, name="c0t")
            wf_d = dram.tile([2 * H, 1], fp32, kind="ExternalInput", uniquify=False, name="wf")
            bf_d = dram.tile([1, 1], fp32, kind="ExternalInput", uniquify=False, name="bf")
            out_d = dram.tile([1, BL], fp32, kind="ExternalOutput", uniquify=False, name="out")

            # ---- constants / weights in SBUF ----
            ident = const.tile([P, P], bf16, name="ident")
            make_identity(nc, ident[:])
            ident_f = const.tile([P, P], fp32, name="ident_f")
            make_identity(nc, ident_f[:])
            xb_sb = const.tile([P, NBLK], i32, name="xb_sb")
            nc.sync.dma_start(out=xb_sb[:], in_=xb_d[:])
            wih_sb = const.tile([E, 4 * H], bf16, name="wih_sb")
            nc.sync.dma_start(out=wih_sb[:], in_=wih_d[:])
            whh_sb = const.tile([H, 4 * H], bf16, name="whh_sb")
            nc.sync.dma_start(out=whh_sb[:], in_=whh_d[:])
            b_sb = const.tile([P, 2], fp32, name="b_sb")
            nc.sync.dma_start(out=b_sb[:], in_=b_d[:].rearrange("a b -> b a"))
            wf_avg = const.tile([H, 1], fp32, name="wf_avg")
            nc.sync.dma_start(out=wf_avg[:], in_=wf_d[0:H, :])
            wf_max = const.tile([H, 1], fp32, name="wf_max")
            nc.sync.dma_start(out=wf_max[:], in_=wf_d[H : 2 * H, :])
            bf_sb = const.tile([1, 1], fp32, name="bf_sb")
            nc.sync.dma_start(out=bf_sb[:], in_=bf_d[:])

            # ---- recurrence state (double buffered) ----
            hT = [state.tile([H, BL], bf16, name=f"hT{i}") for i in range(2)]
            # T2 stack: partitions 0:64 = g_hat, 64:128 = c (f32)
            T2 = [state.tile([P, BL], fp32, name=f"T2{i}") for i in range(2)]
            # S rect: [:, 0:64] = [i_hat | f_hat]; [0:64, 64:128] = o_hat
            S1 = [state.tile([P, P], fp32, name=f"S1{i}") for i in range(2)]
            Ug = [state.tile([H, BL], fp32, name=f"Ug{i}") for i in range(2)]
            Pig = [state.tile([H, BL], fp32, name=f"Pig{i}") for i in range(2)]
            Pfc = [state.tile([H, BL], fp32, name=f"Pfc{i}") for i in range(2)]
            max_acc = state.tile([H, BL], fp32, name="max_acc")
            sum_sb = state.tile([H, BL], fp32, name="sum_sb")
            out_sb = state.tile([1, BL], fp32, name="out_sb")
            pool_ps = ps_pool.tile([H, BL], fp32, name="pool_ps")

            nc.sync.dma_start(out=hT[0][:], in_=h0_d[:])
            nc.sync.dma_start(out=T2[0][64:128, :], in_=c0_d[:])

            # xproj per chunk, bf16, interleaved halves: [P, 2, CCOLS]
            xp = [
                xp_pool.tile([P, 2, CCOLS], bf16, name=f"xp_{c}", tag=f"xp_{c}")
                for c in range(NCHUNK)
            ]

            chunk_state = {}

            def emit_gather(ch, blk):
                if blk == 0:
                    chunk_state[ch] = {
                        "xe": xe_pool.tile([P, CHUNK, E], bf16, tag="xe", name="xe"),
                        "xet": xet_pool.tile([P, CCOLS], bf16, tag="xet", name="xet"),
                    }
                xe = chunk_state[ch]["xe"]
                nc.gpsimd.indirect_dma_start(
                    out=xe[:, blk, :],
                    out_offset=None,
                    in_=emb_d[:],
                    in_offset=bass.IndirectOffsetOnAxis(
                        ap=xb_sb[:, ch * CHUNK + blk : ch * CHUNK + blk + 1],
                        axis=0,
                    ),
                )

            def _anchored(inst, anchor):
                if anchor is not None:
                    add_dep_helper(
                        inst.ins, anchor.ins, sync=False,
                        reason="keep chunk prep behind the recurrence",
                    )

            def emit_tr(ch, blk, anchor=None):
                # transpose one gathered 128-token block into its pt quarter
                st = chunk_state[ch]
                if blk % 4 == 0:
                    st[f"pt{blk // 4}"] = ps_tr.tile(
                        [P, 512], bf16, tag="pt", name="pt"
                    )
                pt = st[f"pt{blk // 4}"]
                tr = nc.tensor.transpose(
                    out=pt[:, (blk % 4) * P : (blk % 4 + 1) * P],
                    in_=st["xe"][:, blk, :],
                    identity=ident[:],
                )
                _anchored(tr, anchor)

            def emit_xet(ch, q, anchor=None):
                # evict a filled pt group into xeT (frees the psum bank fast)
                st = chunk_state[ch]
                cp = nc.vector.tensor_copy(
                    out=st["xet"][:, q * 512 : (q + 1) * 512], in_=st[f"pt{q}"][:]
                )
                _anchored(cp, anchor)

            def emit_piece(ch, q, half, anchor=None):
                # project one 512-col piece (one gate half) of xeT into xp
                st = chunk_state[ch]
                xet = st["xet"]
                cs = slice(q * 512, (q + 1) * 512)
                pp = ps_pj.tile([P, 512], fp32, tag="pp")
                mm = nc.tensor.matmul(
                    out=pp[:],
                    lhsT=wih_sb[:, half * P : (half + 1) * P],
                    rhs=xet[:, cs],
                    start=True,
                    stop=True,
                )
                _anchored(mm, anchor)
                nc.vector.tensor_scalar_add(
                    out=xp[ch][:, half, cs],
                    in0=pp[:],
                    scalar1=b_sb[:, half : half + 1],
                )

            def emit_chunk(ch):
                for blk in range(CHUNK):
                    emit_gather(ch, blk)
                for q in range(CHUNK // 4):
                    for blk in range(q * 4, q * 4 + 4):
                        pass
                for blk in range(CHUNK):
                    emit_tr(ch, blk)
                for q in range(CHUNK // 4):
                    emit_xet(ch, q)
                    emit_piece(ch, q, 0)
                    emit_piece(ch, q, 1)

            def emit_pool_mm(t):
                # sum-pool h_{t+1} on PE (accumulates into pool_ps across steps);
                # emitted one step late so it never blocks the W_hh matmuls
                nc.tensor.matmul(
                    out=pool_ps[:], lhsT=ident[0:H, 0:H], rhs=hT[(t + 1) % 2][:],
                    start=(t == 0), stop=(t == T - 1), skip_group_check=True,
                )

            def emit_step(t):
                cur, nxt = t % 2, (t + 1) % 2
                ch = t // STEPS_PER_CHUNK
                tc_ = t % STEPS_PER_CHUNK
                xc = slice(tc_ * BL, (tc_ + 1) * BL)
                ps = ps_g.tile([P, P], fp32, tag="ps")
                # seed gates psum with xproj_t via one bf16 identity matmul:
                # cols 0:64 = half0 ([i|f]), cols 64:128 = half1 ([o|g])
                nc.tensor.matmul(
                    out=ps[:], lhsT=ident[:], rhs=xp[ch][:, :, xc],
                    start=True, stop=False, skip_group_check=True,
                )
                # accumulate W_hh^T h on top (bf16); [o|g] half first so
                # tanh(g) can run on ACT while PE finishes the [i|f] half
                nc.tensor.matmul(
                    out=ps[:, BL:P], lhsT=whh_sb[:, P : 2 * P], rhs=hT[cur][:],
                    start=False, stop=True, skip_group_check=True,
                )
                nc.tensor.matmul(
                    out=ps[:, 0:BL], lhsT=whh_sb[:, 0:P], rhs=hT[cur][:],
                    start=False, stop=True, skip_group_check=True,
                )
                if t > 0:
                    emit_pool_mm(t - 1)
                # tanh(g): ps partitions 64:128, cols 64:128
                nc.scalar.activation(
                    out=T2[cur][0:H, :], in_=ps[H:P, BL:P], func=AF.Tanh
                )
                # sigmoid over the whole rect (sigma(g) region is junk, unread)
                nc.scalar.activation(out=S1[cur][:], in_=ps[:], func=AF.Sigmoid)
                # c' = i*g + f*c (base-aligned pairs; f*c first, it only needs sigmoid)
                nc.vector.tensor_mul(
                    out=Pfc[cur][:], in0=S1[cur][H:P, 0:BL], in1=T2[cur][H:P, :]
                )
                nc.vector.tensor_mul(
                    out=Pig[cur][:], in0=S1[cur][0:H, 0:BL], in1=T2[cur][0:H, :]
                )
                nc.vector.tensor_add(
                    out=T2[nxt][H:P, :], in0=Pig[cur][:], in1=Pfc[cur][:]
                )
                nc.scalar.activation(
                    out=Ug[cur][:], in_=T2[nxt][H:P, :], func=AF.Tanh
                )
                # h' = o * tanh(c')  (bf16 out feeds next matmul)
                hmul = nc.vector.tensor_mul(
                    out=hT[nxt][:], in0=S1[cur][0:H, BL:P], in1=Ug[cur][:]
                )
                step_h[t] = hmul
                # max-pool on DVE
                if t == 0:
                    nc.vector.tensor_copy(out=max_acc[:], in_=hT[nxt][:])
                else:
                    nc.vector.tensor_max(
                        out=max_acc[:], in0=max_acc[:], in1=hT[nxt][:]
                    )

            # Progressive pipeline: only the first 4 blocks of chunk 0 are
            # prepped up front; all remaining gather/transpose/projection
            # work is woven between recurrence steps (dep-anchored two steps
            # back so the scheduler cannot hoist it into the PE stream where
            # a pending gather would stall the queue head).
            step_h = {}
            for blk in range(4):
                emit_gather(0, blk)
            for blk in range(4):
                emit_tr(0, blk)
            emit_xet(0, 0)
            emit_piece(0, 0, 0)
            emit_piece(0, 0, 1)
            for ch in range(NCHUNK):
                for s in range(STEPS_PER_CHUNK):
                    t = ch * STEPS_PER_CHUNK + s
                    emit_step(t)
                    anc = step_h.get(t - 2)
                    if ch == 0:
                        # chunk 0's own remainder
                        if s < 12:
                            emit_gather(0, s + 4)
                        if 1 <= s <= 12:
                            emit_tr(0, s + 3, anchor=anc)
                        if s in (4, 8, 12):
                            emit_xet(0, s // 4, anchor=anc)
                        if s in (5, 6, 13, 14, 21, 22):
                            q0 = (s - 5) // 8 + 1
                            emit_piece(0, q0, (s - 5) % 8, anchor=anc)
                    else:
                        # pieces q=1..3 of this chunk (transposes done last chunk)
                        if s in (5, 6, 13, 14, 21, 22):
                            q0 = (s - 5) // 8 + 1
                            emit_piece(ch, q0, (s - 5) % 8, anchor=anc)
                    if ch + 1 < NCHUNK:
                        if 4 <= s < 20:
                            emit_gather(ch + 1, s - 4)
                        if s >= 16:
                            emit_tr(ch + 1, s - 16, anchor=anc)
                        if s in (20, 23, 27):
                            emit_xet(ch + 1, 0 if s == 20 else (s - 19) // 4, anchor=anc)
                        if s == 31:
                            emit_xet(ch + 1, 3, anchor=anc)
                            emit_piece(ch + 1, 0, 0, anchor=anc)
                            emit_piece(ch + 1, 0, 1, anchor=anc)
            emit_pool_mm(T - 1)

            # final head: out = sigmoid(wf_avg^T @ sum + wf_max^T @ max + bf)
            nc.vector.tensor_copy(out=sum_sb[:], in_=pool_ps[:])
            pf = ps_g.tile([1, BL], fp32, tag="ps")
            nc.tensor.matmul(
                out=pf[:], lhsT=wf_avg[:], rhs=sum_sb[:], start=True, stop=False
            )
            nc.tensor.matmul(
                out=pf[:], lhsT=wf_max[:], rhs=max_acc[:], start=False, stop=True
            )
            nc.scalar.activation(
                out=out_sb[:], in_=pf[:], func=AF.Sigmoid, bias=bf_sb[:, 0:1]
            )
            nc.sync.dma_start(out=out_d[:], in_=out_sb[:])

    nc.compile()
    return nc


def get_module():
    if "nc" not in _CACHE:
        _CACHE["nc"] = _build_module()
    return _CACHE["nc"]


def make_in_maps(x, h0, c0, emb, W_ih, W_hh, b_lstm, W1, b1, W2, b2):
    """Host-side sharding/layout prep. Returns list of 8 per-core input dicts."""
    import ml_dtypes

    bf16 = ml_dtypes.bfloat16
    x = np.asarray(x)
    h0 = np.asarray(h0, dtype=np.float32)
    c0 = np.asarray(c0, dtype=np.float32)
    emb = np.ascontiguousarray(np.asarray(emb, dtype=np.float32)).astype(bf16)
    W_ih = np.asarray(W_ih, dtype=np.float32)
    W_hh = np.asarray(W_hh, dtype=np.float32)
    b_lstm = np.asarray(b_lstm, dtype=np.float32)
    W1 = np.asarray(W1, dtype=np.float32)
    b1 = np.asarray(b1, dtype=np.float32)
    W2 = np.asarray(W2, dtype=np.float32)
    b2 = np.asarray(b2, dtype=np.float32)

    # gate order [i, f, g, o] -> [i, f, o, g]
    perm = np.concatenate([np.arange(0, 2 * H), np.arange(3 * H, 4 * H),
                           np.arange(2 * H, 3 * H)])
    wih_p = np.ascontiguousarray(W_ih[:, perm]).astype(bf16)
    whh_p = np.ascontiguousarray(W_hh[:, perm]).astype(bf16)
    b_p = np.ascontiguousarray(b_lstm[perm].reshape(2, P))

    wf = (W1 @ W2).astype(np.float32).copy()      # [128, 1]
    wf[:H] /= float(T)                             # fold mean-pool scale
    bf_ = (b1 @ W2 + b2).astype(np.float32).reshape(1, 1)

    in_maps = []
    for c in range(NCORES):
        xl = x[c * BL : (c + 1) * BL].astype(np.int32)      # [64, 256]
        tmaj = np.ascontiguousarray(xl.T).reshape(-1)       # token id (t*BL + b)
        xb = np.ascontiguousarray(tmaj.reshape(NBLK, P).T)  # [128, 128] part-major
        in_maps.append(
            {
                "xb": xb,
                "emb": emb,
                "wih": wih_p,
                "whh": whh_p,
                "blstm": b_p,
                "h0t": np.ascontiguousarray(h0[c * BL : (c + 1) * BL].T).astype(bf16),
                "c0t": np.ascontiguousarray(c0[c * BL : (c + 1) * BL].T),
                "wf": wf,
                "bf": bf_,
            }
        )
    return in_maps


def run_on_cores(nc, in_maps, **kw):
    from concourse import bass_utils
    from concourse.bass_interp import get_hw_module

    old_m = nc.m
    nc.m = get_hw_module(nc.m)
    try:
        return bass_utils.run_bass_kernel_spmd(
            nc, in_maps, core_ids=list(range(len(in_maps))), **kw
        )
    finally:
        nc.m = old_m


def kernel(**inputs):
    in_maps = make_in_maps(**inputs)
    nc = get_module()
    res = run_on_cores(nc, in_maps)
    outs = [np.asarray(r["out"], dtype=np.float32).reshape(BL, 1) for r in res.results]
    return np.concatenate(outs, axis=0)


# revision 27
# speedup vs baseline: 1.1005x; 1.1005x over previous
"""Trainium2 Bass kernel for BCModel: Embedding -> LSTM -> mean/max pool -> MLP -> sigmoid.

Sharding: data-parallel over batch. B=512 split as 64 rows per core across 8 cores.
Weights/embedding table replicated. No collectives needed (forward only).

Per-core pipeline:
  1. indirect-DMA gather of embedding rows (f32, token-partition layout, one
     128-row DMA per block: HW DGE supports one offset per partition)
  2. PE transpose -> xe^T [E=128, tok] layout, evicted to bf16
  3. projection matmul xproj = W_ih^T @ xe^T + b (bf16 x bf16 -> f32), all
     timesteps up front, stored f32 interleaved [128, 2, cols] per chunk
  4. 256-step LSTM recurrence, transposed layout (partition = feature,
     free = batch), gate order [i, f, o, g]:
       - gate psum [128, 128] (one bank): cols 0:64 = [i|f], 64:128 = [o|g]
       - seeded with xproj_t by DMA, W_hh^T h accumulated on top (bf16, h bf16)
       - sigmoid over the whole rect (junk where g sits), tanh(g), tanh(c')
       - c stays f32; sum-pool accumulated on PE (identity fold), max on DVE
  5. final head: out = sigmoid(wf_avg^T sum + wf_max^T max + bf)
"""

import numpy as np

B, T, E, H, VOCAB = 512, 256, 128, 64, 50000
NCORES = 8
BL = B // NCORES          # 64 batch rows per core
P = 128
NBLK = (BL * T) // P      # 128 gather blocks of 128 tokens (2 timesteps each)
CHUNK = 16                # blocks per chunk (2048 tokens)
NCHUNK = NBLK // CHUNK    # 8
CCOLS = CHUNK * P         # 2048 xeT columns per chunk
STEPS_PER_CHUNK = T // NCHUNK  # 32

_CACHE = {}


def _build_module():
    import concourse.bass as bass
    import concourse.mybir as mybir
    import concourse.tile as tile
    from concourse import bacc
    from concourse.masks import make_identity
    from concourse.tile_rust import add_dep_helper

    fp32 = mybir.dt.float32
    bf16 = mybir.dt.bfloat16
    i32 = mybir.dt.int32
    AF = mybir.ActivationFunctionType

    nc = bacc.Bacc(None, target_bir_lowering=False, debug=False)

    with tile.TileContext(nc) as tc:
        with (
            tc.tile_pool(name="dram", bufs=1, space="DRAM") as dram,
            tc.tile_pool(name="const", bufs=1) as const,
            tc.tile_pool(name="xe_pool", bufs=2) as xe_pool,
            tc.tile_pool(name="xet_pool", bufs=2) as xet_pool,
            tc.tile_pool(name="xp_pool", bufs=1) as xp_pool,
            tc.tile_pool(name="state", bufs=1) as state,
            tc.tile_pool(name="ps_tr", bufs=2, space="PSUM") as ps_tr,
            tc.tile_pool(name="ps_pj", bufs=2, space="PSUM") as ps_pj,
            tc.tile_pool(name="ps_g", bufs=2, space="PSUM") as ps_g,
            tc.tile_pool(name="ps_pool", bufs=1, space="PSUM") as ps_pool,
        ):
            # ---- DRAM I/O ----
            xb_d = dram.tile([P, NBLK], i32, kind="ExternalInput", uniquify=False, name="xb")
            emb_d = dram.tile([VOCAB, E], bf16, kind="ExternalInput", uniquify=False, name="emb")
            wih_d = dram.tile([E, 4 * H], bf16, kind="ExternalInput", uniquify=False, name="wih")
            whh_d = dram.tile([H, 4 * H], bf16, kind="ExternalInput", uniquify=False, name="whh")
            b_d = dram.tile([2, P], fp32, kind="ExternalInput", uniquify=False, name="blstm")
            h0_d = dram.tile([H, BL], bf16, kind="ExternalInput", uniquify=False, name="h0t")
            c0_d = dram.tile([H, BL], fp32, kind="ExternalInput", uniquify=False, name="c0t")
            wf_d = dram.tile([2 * H, 1], fp32, kind="ExternalInput", uniquify=False, name="wf")
            bf_d = dram.tile([1, 1], fp32, kind="ExternalInput", uniquify=False, name="bf")
            out_d = dram.tile([1, BL], fp32, kind="ExternalOutput", uniquify=False, name="out")

            # ---- constants / weights in SBUF ----
            ident = const.tile([P, P], bf16, name="ident")
            make_identity(nc, ident[:])
            ident_f = const.tile([P, P], fp32, name="ident_f")
            make_identity(nc, ident_f[:])
            xb_sb = const.tile([P, NBLK], i32, name="xb_sb")
            nc.sync.dma_start(out=xb_sb[:], in_=xb_d[:])
            wih_sb = const.tile([E, 4 * H], bf16, name="wih_sb")
            nc.sync.dma_start(out=wih_sb[:], in_=wih_d[:])
            whh_sb = const.tile([H, 4 * H], bf16, name="whh_sb")
            nc.sync.dma_start(out=whh_sb[:], in_=whh_d[:])
            b_sb = const.tile([P, 2], fp32, name="b_sb")
            nc.sync.dma_start(out=b_sb[:], in_=b_d[:].rearrange("a b -> b a"))
            wf_avg = const.tile([H, 1], fp32, name="wf_avg")
            nc.sync.dma_start(out=wf_avg[:], in_=wf_d[0:H, :])
            wf_max = const.tile([H, 1], fp32, name="wf_max")
            nc.sync.dma_start(out=wf_max[:], in_=wf_d[H : 2 * H, :])
            bf_sb = const.tile([1, 1], fp32, name="bf_sb")
            nc.sync.dma_start(out=bf_sb[:], in_=bf_d[:])

            # ---- recurrence state (double buffered) ----
            hT = [state.tile([H, BL], bf16, name=f"hT{i}") for i in range(2)]
            # T2 stack: partitions 0:64 = g_hat, 64:128 = c (f32)
            T2 = [state.tile([P, BL], fp32, name=f"T2{i}") for i in range(2)]
            # S rect: [:, 0:64] = [i_hat | f_hat]; [0:64, 64:128] = o_hat
            S1 = [state.tile([P, P], fp32, name=f"S1{i}") for i in range(2)]
            Ug = [state.tile([H, BL], fp32, name=f"Ug{i}") for i in range(2)]
            Pig = [state.tile([H, BL], fp32, name=f"Pig{i}") for i in range(2)]
            Pfc = [state.tile([H, BL], fp32, name=f"Pfc{i}") for i in range(2)]
            max_acc = state.tile([H, BL], fp32, name="max_acc")
            sum_sb = state.tile([H, BL], fp32, name="sum_sb")
            out_sb = state.tile([1, BL], fp32, name="out_sb")
            pool_ps = ps_pool.tile([H, BL], fp32, name="pool_ps")

            nc.sync.dma_start(out=hT[0][:], in_=h0_d[:])
            nc.sync.dma_start(out=T2[0][64:128, :], in_=c0_d[:])

            # xproj per chunk, bf16, interleaved halves: [P, 2, CCOLS]
            xp = [
                xp_pool.tile([P, 2, CCOLS], bf16, name=f"xp_{c}", tag=f"xp_{c}")
                for c in range(NCHUNK)
            ]

            chunk_state = {}

            def emit_gather(ch, blk):
                if blk == 0:
                    chunk_state[ch] = {
                        "xe": xe_pool.tile([P, CHUNK, E], bf16, tag="xe", name="xe"),
                        "xet": xet_pool.tile([P, CCOLS], bf16, tag="xet", name="xet"),
                    }
                xe = chunk_state[ch]["xe"]
                nc.gpsimd.indirect_dma_start(
                    out=xe[:, blk, :],
                    out_offset=None,
                    in_=emb_d[:],
                    in_offset=bass.IndirectOffsetOnAxis(
                        ap=xb_sb[:, ch * CHUNK + blk : ch * CHUNK + blk + 1],
                        axis=0,
                    ),
                )

            def _anchored(inst, anchor):
                if anchor is not None:
                    add_dep_helper(
                        inst.ins, anchor.ins, sync=False,
                        reason="keep chunk prep behind the recurrence",
                    )

            def emit_tr(ch, blk, anchor=None):
                # transpose one gathered 128-token block into its pt quarter
                st = chunk_state[ch]
                if blk % 4 == 0:
                    st[f"pt{blk // 4}"] = ps_tr.tile(
                        [P, 512], bf16, tag="pt", name="pt"
                    )
                pt = st[f"pt{blk // 4}"]
                tr = nc.tensor.transpose(
                    out=pt[:, (blk % 4) * P : (blk % 4 + 1) * P],
                    in_=st["xe"][:, blk, :],
                    identity=ident[:],
                )
                _anchored(tr, anchor)

            def emit_xet(ch, q, anchor=None):
                # evict a filled pt group into xeT (frees the psum bank fast)
                st = chunk_state[ch]
                cp = nc.vector.tensor_copy(
                    out=st["xet"][:, q * 512 : (q + 1) * 512], in_=st[f"pt{q}"][:]
                )
                _anchored(cp, anchor)

            def emit_piece(ch, q, half, anchor=None):
                # project one 512-col piece (one gate half) of xeT into xp
                st = chunk_state[ch]
                xet = st["xet"]
                cs = slice(q * 512, (q + 1) * 512)
                pp = ps_pj.tile([P, 512], fp32, tag="pp")
                mm = nc.tensor.matmul(
                    out=pp[:],
                    lhsT=wih_sb[:, half * P : (half + 1) * P],
                    rhs=xet[:, cs],
                    start=True,
                    stop=True,
                )
                _anchored(mm, anchor)
                nc.vector.tensor_scalar_add(
                    out=xp[ch][:, half, cs],
                    in0=pp[:],
                    scalar1=b_sb[:, half : half + 1],
                )

            def emit_chunk(ch):
                for blk in range(CHUNK):
                    emit_gather(ch, blk)
                for q in range(CHUNK // 4):
                    for blk in range(q * 4, q * 4 + 4):
                        pass
                for blk in range(CHUNK):
                    emit_tr(ch, blk)
                for q in range(CHUNK // 4):
                    emit_xet(ch, q)
                    emit_piece(ch, q, 0)
                    emit_piece(ch, q, 1)

            def emit_pool_mm(t):
                # sum-pool h_{t+1} on PE (accumulates into pool_ps across steps);
                # emitted one step late so it never blocks the W_hh matmuls
                nc.tensor.matmul(
                    out=pool_ps[:], lhsT=ident[0:H, 0:H], rhs=hT[(t + 1) % 2][:],
                    start=(t == 0), stop=(t == T - 1), skip_group_check=True,
                )

            def emit_step(t):
                cur, nxt = t % 2, (t + 1) % 2
                ch = t // STEPS_PER_CHUNK
                tc_ = t % STEPS_PER_CHUNK
                xc = slice(tc_ * BL, (tc_ + 1) * BL)
                ps = ps_g.tile([P, P], fp32, tag="ps")
                # seed gates psum with xproj_t via one bf16 identity matmul:
                # cols 0:64 = half0 ([i|f]), cols 64:128 = half1 ([o|g])
                nc.tensor.matmul(
                    out=ps[:], lhsT=ident[:], rhs=xp[ch][:, :, xc],
                    start=True, stop=False, skip_group_check=True,
                )
                # accumulate W_hh^T h on top (bf16)
                nc.tensor.matmul(
                    out=ps[:, 0:BL], lhsT=whh_sb[:, 0:P], rhs=hT[cur][:],
                    start=False, stop=True, skip_group_check=True,
                )
                nc.tensor.matmul(
                    out=ps[:, BL:P], lhsT=whh_sb[:, P : 2 * P], rhs=hT[cur][:],
                    start=False, stop=True, skip_group_check=True,
                )
                if t > 0:
                    emit_pool_mm(t - 1)
                # sigmoid over the whole rect (sigma(g) region is junk, unread)
                nc.scalar.activation(out=S1[cur][:], in_=ps[:], func=AF.Sigmoid)
                # tanh(g): ps partitions 64:128, cols 64:128
                nc.scalar.activation(
                    out=T2[cur][0:H, :], in_=ps[H:P, BL:P], func=AF.Tanh
                )
                # c' = i*g + f*c (base-aligned pairs; f*c first, it only needs sigmoid)
                nc.vector.tensor_mul(
                    out=Pfc[cur][:], in0=S1[cur][H:P, 0:BL], in1=T2[cur][H:P, :]
                )
                nc.vector.tensor_mul(
                    out=Pig[cur][:], in0=S1[cur][0:H, 0:BL], in1=T2[cur][0:H, :]
                )
                nc.vector.tensor_add(
                    out=T2[nxt][H:P, :], in0=Pig[cur][:], in1=Pfc[cur][:]
                )
                nc.scalar.activation(
                    out=Ug[cur][:], in_=T2[nxt][H:P, :], func=AF.Tanh
                )
                # h' = o * tanh(c')  (bf16 out feeds next matmul)
                hmul = nc.vector.tensor_mul(
                    out=hT[nxt][:], in0=S1[cur][0:H, BL:P], in1=Ug[cur][:]
                )
                step_h[t] = hmul
                # max-pool on DVE
                if t == 0:
                    nc.vector.tensor_copy(out=max_acc[:], in_=hT[nxt][:])
                else:
                    nc.vector.tensor_max(
                        out=max_acc[:], in0=max_acc[:], in1=hT[nxt][:]
                    )

            # Progressive pipeline: only the first 4 blocks of chunk 0 are
            # prepped up front; all remaining gather/transpose/projection
            # work is woven between recurrence steps (dep-anchored two steps
            # back so the scheduler cannot hoist it into the PE stream where
            # a pending gather would stall the queue head).
            step_h = {}
            for blk in range(4):
                emit_gather(0, blk)
            for blk in range(4):
                emit_tr(0, blk)
            emit_xet(0, 0)
            emit_piece(0, 0, 0)
            emit_piece(0, 0, 1)
            for ch in range(NCHUNK):
                for s in range(STEPS_PER_CHUNK):
                    t = ch * STEPS_PER_CHUNK + s
                    emit_step(t)
                    anc = step_h.get(t - 2)
                    if ch == 0:
                        # chunk 0's own remainder
                        if s < 12:
                            emit_gather(0, s + 4)
                        if 1 <= s <= 12:
                            emit_tr(0, s + 3, anchor=anc)
                        if s in (4, 8, 12):
                            emit_xet(0, s // 4, anchor=anc)
                        if s in (5, 6, 13, 14, 21, 22):
                            q0 = (s - 5) // 8 + 1
                            emit_piece(0, q0, (s - 5) % 8, anchor=anc)
                    else:
                        # pieces q=1..3 of this chunk (transposes done last chunk)
                        if s in (5, 6, 13, 14, 21, 22):
                            q0 = (s - 5) // 8 + 1
                            emit_piece(ch, q0, (s - 5) % 8, anchor=anc)
                    if ch + 1 < NCHUNK:
                        if 4 <= s < 20:
                            emit_gather(ch + 1, s - 4)
                        if s >= 16:
                            emit_tr(ch + 1, s - 16, anchor=anc)
                        if s in (20, 23, 27):
                            emit_xet(ch + 1, 0 if s == 20 else (s - 19) // 4, anchor=anc)
                        if s == 31:
                            emit_xet(ch + 1, 3, anchor=anc)
                            emit_piece(ch + 1, 0, 0, anchor=anc)
                            emit_piece(ch + 1, 0, 1, anchor=anc)
            emit_pool_mm(T - 1)

            # final head: out = sigmoid(wf_avg^T @ sum + wf_max^T @ max + bf)
            nc.vector.tensor_copy(out=sum_sb[:], in_=pool_ps[:])
            pf = ps_g.tile([1, BL], fp32, tag="ps")
            nc.tensor.matmul(
                out=pf[:], lhsT=wf_avg[:], rhs=sum_sb[:], start=True, stop=False
            )
            nc.tensor.matmul(
                out=pf[:], lhsT=wf_max[:], rhs=max_acc[:], start=False, stop=True
            )
            nc.scalar.activation(
                out=out_sb[:], in_=pf[:], func=AF.Sigmoid, bias=bf_sb[:, 0:1]
            )
            nc.sync.dma_start(out=out_d[:], in_=out_sb[:])

    nc.compile()
    return nc


def get_module():
    if "nc" not in _CACHE:
        _CACHE["nc"] = _build_module()
    return _CACHE["nc"]


def make_in_maps(x, h0, c0, emb, W_ih, W_hh, b_lstm, W1, b1, W2, b2):
    """Host-side sharding/layout prep. Returns list of 8 per-core input dicts."""
    import ml_dtypes

    bf16 = ml_dtypes.bfloat16
    x = np.asarray(x)
    h0 = np.asarray(h0, dtype=np.float32)
    c0 = np.asarray(c0, dtype=np.float32)
    emb = np.ascontiguousarray(np.asarray(emb, dtype=np.float32)).astype(bf16)
    W_ih = np.asarray(W_ih, dtype=np.float32)
    W_hh = np.asarray(W_hh, dtype=np.float32)
    b_lstm = np.asarray(b_lstm, dtype=np.float32)
    W1 = np.asarray(W1, dtype=np.float32)
    b1 = np.asarray(b1, dtype=np.float32)
    W2 = np.asarray(W2, dtype=np.float32)
    b2 = np.asarray(b2, dtype=np.float32)

    # gate order [i, f, g, o] -> [i, f, o, g]
    perm = np.concatenate([np.arange(0, 2 * H), np.arange(3 * H, 4 * H),
                           np.arange(2 * H, 3 * H)])
    wih_p = np.ascontiguousarray(W_ih[:, perm]).astype(bf16)
    whh_p = np.ascontiguousarray(W_hh[:, perm]).astype(bf16)
    b_p = np.ascontiguousarray(b_lstm[perm].reshape(2, P))

    wf = (W1 @ W2).astype(np.float32).copy()      # [128, 1]
    wf[:H] /= float(T)                             # fold mean-pool scale
    bf_ = (b1 @ W2 + b2).astype(np.float32).reshape(1, 1)

    in_maps = []
    for c in range(NCORES):
        xl = x[c * BL : (c + 1) * BL].astype(np.int32)      # [64, 256]
        tmaj = np.ascontiguousarray(xl.T).reshape(-1)       # token id (t*BL + b)
        xb = np.ascontiguousarray(tmaj.reshape(NBLK, P).T)  # [128, 128] part-major
        in_maps.append(
            {
                "xb": xb,
                "emb": emb,
                "wih": wih_p,
                "whh": whh_p,
                "blstm": b_p,
                "h0t": np.ascontiguousarray(h0[c * BL : (c + 1) * BL].T).astype(bf16),
                "c0t": np.ascontiguousarray(c0[c * BL : (c + 1) * BL].T),
                "wf": wf,
                "bf": bf_,
            }
        )
    return in_maps


def run_on_cores(nc, in_maps, **kw):
    from concourse import bass_utils
    from concourse.bass_interp import get_hw_module

    old_m = nc.m
    nc.m = get_hw_module(nc.m)
    try:
        return bass_utils.run_bass_kernel_spmd(
            nc, in_maps, core_ids=list(range(len(in_maps))), **kw
        )
    finally:
        nc.m = old_m


def kernel(**inputs):
    in_maps = make_in_maps(**inputs)
    nc = get_module()
    res = run_on_cores(nc, in_maps)
    outs = [np.asarray(r["out"], dtype=np.float32).reshape(BL, 1) for r in res.results]
    return np.concatenate(outs, axis=0)
